# revision 2
# baseline (speedup 1.0000x reference)
"""Trainium2 Bass kernel for nn_MistralAttention_KVmix (v3: wave-pipelined).

Same algorithm/layout as v2 (112x64 contiguous quant part, transposed
scores, bf16 matmul operands, fp32 quant math), but K and V are processed
in half-waves (32 positions per partition each) with double-buffered
staging so batch b+1's DMA/transposes/stats overlap batch b's dequant,
and the QK/PV matmuls are emitted inside the waves.  o_proj and the
projections run bf16 (ACT-cast weights) to shrink the serial tail.
"""

import os
import sys

import numpy as np
import ml_dtypes

_BF = ml_dtypes.bfloat16

for _p in ("/opt/trn_rl_repo",):
    if os.path.isdir(_p) and _p not in sys.path:
        sys.path.insert(0, _p)

import concourse.bass as bass
import concourse.mybir as mybir
import concourse.tile as tile
from concourse.bass_utils import run_bass_kernel_spmd

F32 = mybir.dt.float32
BF16 = mybir.dt.bfloat16
AX = mybir.AxisListType
OP = mybir.AluOpType
ACTF = mybir.ActivationFunctionType

B = 4
NH = 4
D = 128
S = 8192
NQ = 7168
P1 = 112          # partitions, quant part; s = 64*p + t
T1 = 64
E = 32            # wave size (half of T1) == quant group size
PF = 128          # tail partitions; s = NQ + 8*p + t
TF = 8
MAGIC = 8388608.0
INV_SQRT_D = float(1.0 / np.sqrt(np.float32(D)))
C1 = 6.28125
C2 = float(np.float32(2.0 * np.pi - 6.28125))
INV_2PI = float(np.float32(1.0 / (2.0 * np.pi)))

DEBUG_DUMPS = False

# which big elementwise passes run on GpSimd instead of DVE (tunable)
GPS_PASSES = {"k1": False, "k2": False, "k3": False,
              "v1": True, "v2": True, "v3": False}


def _bc(ap, axis, n):
    shape = list(ap.shape)
    shape.insert(axis, n)
    return ap.unsqueeze(axis).to_broadcast(tuple(shape))


def _split_multi_waits(nc):
    """walrus encodes at most one semaphore wait per TPB instruction; split
    multi-wait instructions by inserting same-engine NOPs carrying the
    extra waits."""
    for f in nc.m.functions:
        blocks = list(f.blocks)
        for blk in blocks:
            live = blk.instructions
            orig = list(live)
            new = []
            changed = False
            for inst in orig:
                si = inst.sync_info
                waits = list(si.on_wait) if (si and si.on_wait) else []
                if len(waits) > 1 and inst.engine != mybir.EngineType.Unassigned:
                    eng = nc.engines[inst.engine]
                    for w in waits[:-1]:
                        nop = eng.drain().ins
                        for b2 in f.blocks:
                            l2 = b2.instructions
                            if l2 and l2[-1] is nop:
                                l2.pop()
                                break
                        nop.sync_info = mybir.SyncInfo(on_wait=[w],
                                                       on_update=[])
                        new.append(nop)
                    inst.sync_info = mybir.SyncInfo(
                        on_wait=[waits[-1]],
                        on_update=list(si.on_update or []))
                    changed = True
                new.append(inst)
            if changed:
                live[:] = new


def build_nc():
    nc = bass.Bass()

    hidden = nc.declare_dram_parameter("hidden", [B, 4096], F32, isOutput=False)
    kp = nc.declare_dram_parameter("kp", [B, S, D], F32, isOutput=False)
    vp = nc.declare_dram_parameter("vp", [B, S, D], F32, isOutput=False)
    wq = nc.declare_dram_parameter("wq", [4096, NH * D], BF16, isOutput=False)
    wk = nc.declare_dram_parameter("wk", [4096, D], BF16, isOutput=False)
    wv = nc.declare_dram_parameter("wv", [4096, D], BF16, isOutput=False)
    wo = nc.declare_dram_parameter("wo", [NH * D, 4096], BF16, isOutput=False)
    pos = nc.declare_dram_parameter("pos", [1, B], F32, isOutput=False)
    ident = nc.declare_dram_parameter("ident", [128, 128], F32, isOutput=False)
    invf = nc.declare_dram_parameter("invf", [128, 1], F32, isOutput=False)
    sgn = nc.declare_dram_parameter("sgn", [128, 1], F32, isOutput=False)
    out_d = nc.declare_dram_parameter("out", [B, 4096], F32, isOutput=True)

    with tile.TileContext(nc) as tc:
        _emit(nc, tc, hidden, kp, vp, wq, wk, wv, wo, pos, ident, invf, sgn,
              out_d)
    _split_multi_waits(nc)
    return nc


def _emit(nc, tc, hidden, kp, vp, wq, wk, wv, wo, pos, ident, invf, sgn,
          out_d):
    from contextlib import ExitStack

    def eng(key):
        return nc.gpsimd if GPS_PASSES[key] else nc.vector

    with ExitStack() as ctx:
        ec = ctx.enter_context
        singles = ec(tc.tile_pool(name="singles", bufs=1))
        kcp = ec(tc.tile_pool(name="kcp", bufs=2))
        vcp = ec(tc.tile_pool(name="vcp", bufs=2))
        ktp = ec(tc.tile_pool(name="ktp", bufs=2))
        kzp = ec(tc.tile_pool(name="kzp", bufs=2))
        vzp = ec(tc.tile_pool(name="vzp", bufs=2))
        tails = ec(tc.tile_pool(name="tails", bufs=1))
        stats = ec(tc.tile_pool(name="stats", bufs=2))
        wpool = ec(tc.tile_pool(name="wpool", bufs=4))
        ptp = ec(tc.tile_pool(name="ptp", bufs=2))
        misc = ec(tc.tile_pool(name="misc", bufs=2))
        hstp = ec(tc.tile_pool(name="hstp", bufs=1))
        wop = ec(tc.tile_pool(name="wop", bufs=8))
        pstr = ec(tc.tile_pool(name="pstr", bufs=2, space="PSUM"))
        psatt = ec(tc.tile_pool(name="psatt", bufs=2, space="PSUM"))
        pspv = ec(tc.tile_pool(name="pspv", bufs=2, space="PSUM"))
        pssm = ec(tc.tile_pool(name="pssm", bufs=2, space="PSUM"))

        # ---- constants -------------------------------------------------
        ident_sb = singles.tile([128, 128], F32)
        nc.sync.dma_start(out=ident_sb[:], in_=ident[:])
        invf_sb = singles.tile([128, 1], F32)
        nc.sync.dma_start(out=invf_sb[:], in_=invf[:])
        sgn_sb = singles.tile([128, 1], F32)
        nc.sync.dma_start(out=sgn_sb[:], in_=sgn[:])
        posr = singles.tile([128, B], F32)
        nc.sync.dma_start(out=posr[:], in_=pos[:].to_broadcast((128, B)))
        zerob = singles.tile([128, 1], F32)
        nc.vector.memset(zerob[:], 0.0)
        ones1 = singles.tile([P1, 1], BF16)
        nc.vector.memset(ones1[:], 1.0)
        onesF = singles.tile([PF, 1], BF16)
        nc.vector.memset(onesF[:], 1.0)

        # ---- kick off b=0 cache loads before anything else -------------
        kcs = {}
        vcs = {}

        def load_k_wave(b, u):
            kc = kcp.tile([P1, E, D], F32, tag="kc")
            nc.sync.dma_start(
                out=kc[:],
                in_=kp[b, 0:NQ, :].rearrange("(p t) d -> p t d",
                                             p=P1)[:, E * u:E * (u + 1), :])
            return kc

        def load_v_wave(b, w):
            vc = vcp.tile([P1, E, D], F32, tag="vc")
            nc.sync.dma_start(
                out=vc[:],
                in_=vp[b, 0:NQ, :].rearrange("(p t) d -> p t d",
                                             p=P1)[:, E * w:E * (w + 1), :])
            return vc

        kcs[(0, 0)] = load_k_wave(0, 0)
        vcs[(0, 0)] = load_v_wave(0, 0)
        kcs[(0, 1)] = load_k_wave(0, 1)
        vcs[(0, 1)] = load_v_wave(0, 1)

        # ---- hidden^T --------------------------------------------------
        hT = singles.tile([128, 32, B], F32)
        for kk in range(0, 32, 8):
            hst = hstp.tile([B, 8 * 128], F32, tag="hst")
            nc.sync.dma_start(out=hst[:],
                              in_=hidden[:, 1024 * (kk // 8):
                                         1024 * (kk // 8 + 1)])
            ps_h = pssm.tile([128, 8 * B], F32, tag="sm")
            for j in range(8):
                nc.tensor.transpose(
                    ps_h[:, 4 * j:4 * j + 4],
                    hst[:, 128 * j:128 * (j + 1)],
                    ident_sb[0:B, 0:B],
                )
            nc.scalar.copy(hT[:, kk:kk + 8, :].rearrange("p k b -> p (k b)"),
                           ps_h[:])
        hTb = singles.tile([128, 32, B], BF16)
        nc.scalar.copy(hTb[:].rearrange("p k b -> p (k b)"),
                       hT[:].rearrange("p k b -> p (k b)"))

        # ---- projections (bf16, ACT-cast weights) ----------------------
        q_bh = singles.tile([B, NH * D], F32)
        k_bd = singles.tile([B, D], F32)
        v_new = singles.tile([B, D], F32)
        for w_d, n_cols, dst, wtag in ((wq, NH * D, q_bh, "wq"),
                                       (wk, D, k_bd, "wk"),
                                       (wv, D, v_new, "wv")):
            ps_p = pssm.tile([B, n_cols], F32, tag="sm")
            for k in range(32):
                w_t = wpool.tile([128, n_cols], BF16, tag=wtag)
                nc.sync.dma_start(out=w_t[:],
                                  in_=w_d[128 * k:128 * (k + 1), :])
                nc.tensor.matmul(ps_p[:], hTb[:, k, :], w_t[:],
                                 start=(k == 0), stop=(k == 31))
            nc.scalar.copy(dst[:], ps_p[:])
        v_new_f = singles.tile([1, B, D], F32)
        for bb in range(B):
            nc.sync.dma_start(out=v_new_f[0:1, bb, :],
                              in_=v_new[bb:bb + 1, :])
        v_new_b = singles.tile([1, B, D], BF16)
        nc.scalar.copy(v_new_b[0:1, :, :].rearrange("p b d -> p (b d)"),
                       v_new_f[0:1, :, :].rearrange("p b d -> p (b d)"))

        ps_qT = pssm.tile([128, NH * B], F32, tag="sm")
        for h in range(NH):
            nc.tensor.transpose(ps_qT[:, 4 * h:4 * h + 4],
                                q_bh[:, 128 * h:128 * (h + 1)],
                                ident_sb[0:B, 0:B])
        qT = singles.tile([128, NH, B], F32)
        nc.scalar.copy(qT[:].rearrange("p h b -> p (h b)"), ps_qT[:])
        ps_kT = pssm.tile([128, B], F32, tag="sm")
        nc.tensor.transpose(ps_kT[:], k_bd[:], ident_sb[0:B, 0:B])
        kT_new = singles.tile([128, B], F32)
        nc.scalar.copy(kT_new[:], ps_kT[:])

        # ---- RoPE ------------------------------------------------------
        fT = singles.tile([128, B], F32)
        nc.vector.tensor_mul(fT[:], posr[:], invf_sb[:].to_broadcast((128, B)))
        rk = singles.tile([128, B], F32)
        nc.vector.tensor_scalar(rk[:], fT[:], INV_2PI, None, OP.mult)
        nc.vector.tensor_scalar(rk[:], rk[:], MAGIC * 1.5, MAGIC * 1.5,
                                OP.add, OP.subtract)
        m1 = singles.tile([128, B], F32)
        nc.vector.scalar_tensor_tensor(m1[:], rk[:], -C1, fT[:],
                                       OP.mult, OP.add)
        nc.vector.scalar_tensor_tensor(m1[:], rk[:], -C2, m1[:],
                                       OP.mult, OP.add)
        sinT = singles.tile([128, B], F32)
        cosT = singles.tile([128, B], F32)
        nc.scalar.activation(sinT[:], m1[:], ACTF.Sin, bias=zerob[:])
        fc = singles.tile([128, B], F32)
        nc.vector.tensor_scalar(fc[:], fT[:], float(np.pi / 2), None, OP.add)
        rkc = singles.tile([128, B], F32)
        nc.vector.tensor_scalar(rkc[:], fc[:], INV_2PI, None, OP.mult)
        nc.vector.tensor_scalar(rkc[:], rkc[:], MAGIC * 1.5, MAGIC * 1.5,
                                OP.add, OP.subtract)
        mc = singles.tile([128, B], F32)
        nc.vector.scalar_tensor_tensor(mc[:], rkc[:], -C1, fc[:],
                                       OP.mult, OP.add)
        nc.vector.scalar_tensor_tensor(mc[:], rkc[:], -C2, mc[:],
                                       OP.mult, OP.add)
        nc.scalar.activation(cosT[:], mc[:], ACTF.Sin, bias=zerob[:])
        nc.vector.tensor_scalar(sinT[:], sinT[:], sgn_sb[:], None, OP.mult)

        qsw = singles.tile([128, NH, B], F32)
        nc.sync.dma_start(out=qsw[0:64], in_=qT[64:128])
        nc.sync.dma_start(out=qsw[64:128], in_=qT[0:64])
        ksw = singles.tile([128, B], F32)
        nc.sync.dma_start(out=ksw[0:64], in_=kT_new[64:128])
        nc.sync.dma_start(out=ksw[64:128], in_=kT_new[0:64])

        qR = singles.tile([128, NH, B], F32)
        nc.vector.tensor_mul(qR[:], qT[:], _bc(cosT[:], 1, NH))
        qs2 = singles.tile([128, NH, B], F32)
        nc.vector.tensor_mul(qs2[:], qsw[:], _bc(sinT[:], 1, NH))
        nc.vector.tensor_add(qR[:], qR[:], qs2[:])
        kR = singles.tile([128, B], F32)
        nc.vector.tensor_mul(kR[:], kT_new[:], cosT[:])
        ks2 = singles.tile([128, B], F32)
        nc.vector.tensor_mul(ks2[:], ksw[:], sinT[:])
        nc.vector.tensor_add(kR[:], kR[:], ks2[:])

        qbb = singles.tile([128, NH, B], BF16)
        nc.scalar.copy(qbb[:].rearrange("p h b -> p (h b)"),
                       qR[:].rearrange("p h b -> p (h b)"))
        kRb = singles.tile([128, B], BF16)
        nc.scalar.copy(kRb[:], kR[:])

        oTo = singles.tile([128, NH, B], BF16)

        for b in range(B):
            qb = qbb[:, :, b]

            # ======== V: two dequant half-waves (DVE work first) ========
            vz = vzp.tile([P1, T1, NH + D], BF16)
            for w in range(2):
                vc = vcs.pop((b, w), None)
                if vc is None:
                    vc = load_v_wave(b, w)
                vcv = vc[:].rearrange("p t (g e) -> p t g e", e=32)
                mnV = stats.tile([P1, E, 4], F32, tag="mnV")
                mxV = stats.tile([P1, E, 4], F32, tag="mxV")
                nc.vector.tensor_reduce(mnV[:], vcv, axis=AX.X, op=OP.min)
                nc.vector.tensor_reduce(mxV[:], vcv, axis=AX.X, op=OP.max)
                scV = stats.tile([P1, E, 4], F32, tag="scV")
                scv_f = scV[:].rearrange("p t g -> p (t g)")
                mnv_f = mnV[:].rearrange("p t g -> p (t g)")
                mxv_f = mxV[:].rearrange("p t g -> p (t g)")
                nc.vector.tensor_sub(scv_f, mxv_f, mnv_f)
                nc.vector.tensor_scalar(scv_f, scv_f, 1.0 / 3.0, None,
                                        OP.mult)
                invV = stats.tile([P1, E, 4], F32, tag="invV")
                invv_f = invV[:].rearrange("p t g -> p (t g)")
                nc.vector.reciprocal(invv_f, scv_f)
                vzw = vz[:, E * w:E * (w + 1), :]
                nc.scalar.copy(vzw[:, :, 0:NH], mnV[:])
                vcf = vc[:].rearrange("p t (g e) -> p (t g) e", e=32)
                eng("v1").tensor_sub(vcf, vcf, _bc(mnv_f, 2, 32))
                eng("v2").tensor_mul(vcf, vcf, _bc(invv_f, 2, 32))
                nc.vector.tensor_scalar(
                    vc[:].rearrange("p t d -> p (t d)"),
                    vc[:].rearrange("p t d -> p (t d)"),
                    MAGIC, None, OP.add)
                for g in range(4):
                    eng("v3").scalar_tensor_tensor(
                        vzw[:, :, NH + 32 * g:NH + 32 * (g + 1)],
                        vc[:, :, 32 * g:32 * (g + 1)], MAGIC,
                        _bc(scV[:, :, g], 2, 32), OP.subtract, OP.mult)
                nxt = (b, w + 1) if w == 0 else (b + 1, 0)
                if nxt[0] < B and nxt not in vcs:
                    vcs[nxt] = load_v_wave(*nxt)

            # ======== K: two half-waves ========
            kzT = kzp.tile([128, T1, P1], BF16)
            pa = psatt.tile([P1, T1 + 2, NH], F32)
            for u in range(2):
                kc = kcs.pop((b, u), None)
                if kc is None:
                    kc = load_k_wave(b, u)
                kt = ktp.tile([128, E, P1], F32)
                for g4 in range(8):
                    ptk = pstr.tile([128, 4, D], F32, tag="ptr")
                    for j in range(4):
                        t = 4 * g4 + j
                        nc.tensor.transpose(ptk[:, j, 0:P1], kc[:, t, :],
                                            ident_sb[0:P1, 0:P1])
                    nc.scalar.copy(kt[:, 4 * g4:4 * g4 + 4, :],
                                   ptk[:, :, 0:P1])
                # stats: group(2c+u) x d, reduce over e (= t within wave)
                ktv = kt[:].rearrange("p e c -> p c e")
                mnK = stats.tile([128, P1], F32, tag="mnK")
                mxK = stats.tile([128, P1], F32, tag="mxK")
                nc.vector.tensor_reduce(mnK[:], ktv, axis=AX.X, op=OP.min)
                nc.vector.tensor_reduce(mxK[:], ktv, axis=AX.X, op=OP.max)
                scK = stats.tile([128, P1], F32, tag="scK")
                nc.vector.tensor_sub(scK[:], mxK[:], mnK[:])
                nc.vector.tensor_scalar(scK[:], scK[:], 1.0 / 3.0, None,
                                        OP.mult)
                invK = stats.tile([128, P1], F32, tag="invK")
                nc.vector.reciprocal(invK[:], scK[:])
                mnKb = stats.tile([128, P1], BF16, tag="mnKb")
                nc.scalar.copy(mnKb[:], mnK[:])
                # dequant: y=x-mn; t=y*inv; r'=t+2^23 (RNE); z=(r'-M)*sc
                eng("k1").tensor_sub(kt[:], kt[:], _bc(mnK[:], 1, E))
                eng("k2").tensor_mul(kt[:], kt[:], _bc(invK[:], 1, E))
                nc.vector.tensor_scalar(
                    kt[:].rearrange("p e c -> p (e c)"),
                    kt[:].rearrange("p e c -> p (e c)"),
                    MAGIC, None, OP.add)
                eng("k3").scalar_tensor_tensor(
                    kzT[:, E * u:E * (u + 1), :], kt[:], MAGIC,
                    _bc(scK[:], 1, E), OP.subtract, OP.mult)
                # prefetch next wave's load
                nxt = (b, u + 1) if u == 0 else (b + 1, 0)
                if nxt[0] < B and nxt not in kcs:
                    kcs[nxt] = load_k_wave(*nxt)
                # scores for this half + group-mean fold
                for t in range(E * u, E * (u + 1)):
                    nc.tensor.matmul(pa[:, t, :], kzT[:, t, :], qb,
                                     start=True, stop=True)
                nc.tensor.matmul(pa[:, T1 + u, :], mnKb[:], qb,
                                 start=True, stop=True)

            # ======== K tail ========
            kcF = tails.tile([PF, TF, D], F32, tag="kcF")
            nc.sync.dma_start(
                out=kcF[:],
                in_=kp[b, NQ:S, :].rearrange("(p t) d -> p t d", p=PF))
            kTF = tails.tile([128, TF, D], BF16, tag="kTF")
            for g4 in range(2):
                ptf = pstr.tile([128, 4, D], F32, tag="ptr")
                for j in range(4):
                    t = 4 * g4 + j
                    nc.tensor.transpose(ptf[:, j, :], kcF[:, t, :],
                                        ident_sb[:])
                nc.scalar.copy(
                    kTF[:, 4 * g4:4 * g4 + 4, :].rearrange(
                        "p t c -> p (t c)"),
                    ptf[:].rearrange("p t c -> p (t c)"))
            paf = pssm.tile([PF, TF + 1, NH], F32, tag="sm")
            for t in range(TF):
                nc.tensor.matmul(paf[:, t, :], kTF[:, t, :], qb,
                                 start=True, stop=True)
            nc.tensor.matmul(paf[0:1, TF, :], kRb[:, b:b + 1], qb,
                             start=True, stop=True)

            # ======== softmax numerators ========
            pmns = misc.tile([P1, 2, NH], F32, tag="pmns")
            nc.scalar.copy(pmns[:].rearrange("p u h -> p (u h)"),
                           pa[:, T1:T1 + 2, :].rearrange("p u h -> p (u h)"))
            for u in range(2):
                pah = pa[:, E * u:E * (u + 1), :]
                nc.vector.tensor_add(pah, pah, _bc(pmns[:, u, :], 1, E))
            pT = ptp.tile([P1, T1, NH], BF16, tag="pT")
            nc.scalar.activation(pT[:].rearrange("p t h -> p (t h)"),
                                 pa[:, 0:T1, :].rearrange("p t h -> p (t h)"),
                                 ACTF.Exp, bias=zerob[0:P1, :],
                                 scale=INV_SQRT_D)
            pTf = ptp.tile([PF, TF, NH], BF16, tag="pTf")
            nc.scalar.activation(pTf[:].rearrange("p t h -> p (t h)"),
                                 paf[:, 0:TF, :].rearrange("p t h -> p (t h)"),
                                 ACTF.Exp, bias=zerob[0:PF, :],
                                 scale=INV_SQRT_D)
            pTn = ptp.tile([1, NH], BF16, tag="pTn")
            nc.scalar.activation(pTn[:], paf[0:1, TF, :], ACTF.Exp,
                                 bias=zerob[0:1, :], scale=INV_SQRT_D)

            # ======== denominators ========
            psd = pssm.tile([1, (T1 + TF + 1) * NH], F32, tag="sm")
            nc.tensor.matmul(psd[:, 0:T1 * NH], ones1[:],
                             pT[:].rearrange("p t h -> p (t h)"),
                             start=True, stop=True)
            nc.tensor.matmul(psd[:, T1 * NH:(T1 + TF) * NH], onesF[:],
                             pTf[:].rearrange("p t h -> p (t h)"),
                             start=True, stop=True)
            nc.tensor.matmul(psd[:, (T1 + TF) * NH:], ones1[0:1, :], pTn[:],
                             start=True, stop=True)
            stot = misc.tile([1, NH], F32, tag="stot")
            nc.vector.tensor_reduce(
                stot[:],
                psd[:].rearrange("p (t h) -> p h t", h=NH),
                axis=AX.X, op=OP.add)
            pst = pssm.tile([NH, 1], F32, tag="sm")
            nc.tensor.transpose(pst[:], stot[:], ident_sb[0:1, 0:1])
            rsc = misc.tile([NH, 1], F32, tag="rsc")
            nc.vector.reciprocal(rsc[:], pst[:])

            # ======== PV ========
            pv = pspv.tile([NH, NH + D], F32, tag="pv")
            for t in range(T1):
                nc.tensor.matmul(pv[:], pT[:, t, :], vz[:, t, :],
                                 start=(t == 0), stop=False)

            # ======== V tail + PV tail ========
            vcF = tails.tile([PF, TF, D], F32, tag="vcF")
            nc.sync.dma_start(
                out=vcF[:],
                in_=vp[b, NQ:S, :].rearrange("(p t) d -> p t d", p=PF))
            vFb = tails.tile([PF, TF, D], BF16, tag="vFb")
            nc.scalar.copy(vFb[:].rearrange("p t d -> p (t d)"),
                           vcF[:].rearrange("p t d -> p (t d)"))
            for t in range(TF):
                nc.tensor.matmul(pv[:, NH:], pTf[:, t, :], vFb[:, t, :],
                                 start=False, stop=False)
            nc.tensor.matmul(pv[:, NH:], pTn[:], v_new_b[0:1, b, :],
                             start=False, stop=True)
            pvmn = misc.tile([NH, NH], F32, tag="pvmn")
            nc.scalar.copy(pvmn[:], pv[:, 0:NH])
            ob = misc.tile([NH, D], F32, tag="ob")
            nc.vector.scalar_tensor_tensor(
                ob[:].rearrange("h (g e) -> h g e", e=32),
                pv[:, NH:].rearrange("h (g e) -> h g e", e=32),
                0.0, _bc(pvmn[:], 2, 32), OP.add, OP.add)
            obs = misc.tile([NH, D], F32, tag="obs")
            nc.scalar.activation(obs[:], ob[:], ACTF.Copy, scale=rsc[:])
            pso2 = pssm.tile([128, NH], F32, tag="sm")
            nc.tensor.transpose(pso2[:], obs[:], ident_sb[0:NH, 0:NH])
            nc.scalar.copy(oTo[:, :, b], pso2[:])

            # ---- o_proj for this batch row (overlaps later batches) ----
            for nch in range(8):
                pso = pspv.tile([B, 512], F32, tag="pv")
                for h in range(NH):
                    wo_t = wop.tile([128, 512], BF16, tag="wo")
                    nc.sync.dma_start(
                        out=wo_t[:],
                        in_=wo[128 * h:128 * (h + 1),
                               512 * nch:512 * (nch + 1)])
                    nc.tensor.matmul(pso[0:1, :], oTo[:, h, b:b + 1], wo_t[:],
                                     start=(h == 0), stop=(h == NH - 1))
                outp = misc.tile([B, 512], F32, tag="outp")
                nc.scalar.copy(outp[0:1, :], pso[0:1, :])
                nc.sync.dma_start(
                    out=out_d[b:b + 1, 512 * nch:512 * (nch + 1)],
                    in_=outp[0:1, :])




# ----------------------------------------------------------------------
_NC = None


def _get_nc():
    global _NC
    if _NC is None:
        _NC = build_nc()
    return _NC


def _host_consts():
    ident = np.eye(128, dtype=np.float32)
    inv_freq = (1.0 / (np.float32(10000.0) **
                       (np.arange(0, D, 2).astype(np.float32) / np.float32(D))))
    invf = np.tile(inv_freq.astype(np.float32), 2).reshape(128, 1)
    sgn = np.concatenate([-np.ones(64, np.float32),
                          np.ones(64, np.float32)]).reshape(128, 1)
    return ident, invf, sgn


def _make_in_maps(hidden_states, key_past, value_past, wq, wk, wv, wo,
                  position_ids, past_len=None):
    ident, invf, sgn = _host_consts()
    pos_f = np.asarray(position_ids).astype(np.float32).reshape(1, B)
    hid = np.ascontiguousarray(
        np.asarray(hidden_states, np.float32).reshape(B, 4096))
    in_maps = []
    for c in range(8):
        in_maps.append({
            "hidden": hid,
            "kp": np.ascontiguousarray(np.asarray(key_past, np.float32)[:, c]),
            "vp": np.ascontiguousarray(np.asarray(value_past, np.float32)[:, c]),
            "wq": np.ascontiguousarray(np.asarray(wq, np.float32)[:, 512 * c:512 * (c + 1)].astype(_BF)),
            "wk": np.ascontiguousarray(np.asarray(wk, np.float32)[:, 128 * c:128 * (c + 1)].astype(_BF)),
            "wv": np.ascontiguousarray(np.asarray(wv, np.float32)[:, 128 * c:128 * (c + 1)].astype(_BF)),
            "wo": np.ascontiguousarray(np.asarray(wo, np.float32)[512 * c:512 * (c + 1), :].astype(_BF)),
            "pos": pos_f,
            "ident": ident,
            "invf": invf,
            "sgn": sgn,
        })
    return in_maps


def kernel(hidden_states, key_past, value_past, wq, wk, wv, wo, position_ids,
           past_len):
    nc = _get_nc()
    in_maps = _make_in_maps(hidden_states, key_past, value_past, wq, wk, wv,
                            wo, position_ids)
    res = run_bass_kernel_spmd(nc, in_maps, list(range(8)))
    out = np.zeros((B, 4096), np.float32)
    for r in res.results:
        out = out + r["out"]
    return out.reshape(B, 1, 4096)


# revision 5
# speedup vs baseline: 1.0008x; 1.0008x over previous
"""Trainium2 Bass kernel for nn_MistralAttention_KVmix (v3: wave-pipelined).

Same algorithm/layout as v2 (112x64 contiguous quant part, transposed
scores, bf16 matmul operands, fp32 quant math), but K and V are processed
in half-waves (32 positions per partition each) with double-buffered
staging so batch b+1's DMA/transposes/stats overlap batch b's dequant,
and the QK/PV matmuls are emitted inside the waves.  o_proj and the
projections run bf16 (ACT-cast weights) to shrink the serial tail.
"""

import os
import sys

import numpy as np
import ml_dtypes

_BF = ml_dtypes.bfloat16

for _p in ("/opt/trn_rl_repo",):
    if os.path.isdir(_p) and _p not in sys.path:
        sys.path.insert(0, _p)

import concourse.bass as bass
import concourse.mybir as mybir
import concourse.tile as tile
from concourse.bass_utils import run_bass_kernel_spmd

F32 = mybir.dt.float32
BF16 = mybir.dt.bfloat16
AX = mybir.AxisListType
OP = mybir.AluOpType
ACTF = mybir.ActivationFunctionType

B = 4
NH = 4
D = 128
S = 8192
NQ = 7168
P1 = 112          # partitions, quant part; s = 64*p + t
T1 = 64
E = 32            # wave size (half of T1) == quant group size
PF = 128          # tail partitions; s = NQ + 8*p + t
TF = 8
MAGIC = 8388608.0
INV_SQRT_D = float(1.0 / np.sqrt(np.float32(D)))
C1 = 6.28125
C2 = float(np.float32(2.0 * np.pi - 6.28125))
INV_2PI = float(np.float32(1.0 / (2.0 * np.pi)))

DEBUG_DUMPS = False

# which big elementwise passes run on GpSimd instead of DVE (tunable)
GPS_PASSES = {"k1": False, "k2": False, "k3": False,
              "v1": True, "v2": True, "v3": False}


def _bc(ap, axis, n):
    shape = list(ap.shape)
    shape.insert(axis, n)
    return ap.unsqueeze(axis).to_broadcast(tuple(shape))


def _split_multi_waits(nc):
    """walrus encodes at most one semaphore wait per TPB instruction; split
    multi-wait instructions by inserting same-engine NOPs carrying the
    extra waits."""
    for f in nc.m.functions:
        blocks = list(f.blocks)
        for blk in blocks:
            live = blk.instructions
            orig = list(live)
            new = []
            changed = False
            for inst in orig:
                si = inst.sync_info
                waits = list(si.on_wait) if (si and si.on_wait) else []
                if len(waits) > 1 and inst.engine != mybir.EngineType.Unassigned:
                    eng = nc.engines[inst.engine]
                    for w in waits[:-1]:
                        nop = eng.drain().ins
                        for b2 in f.blocks:
                            l2 = b2.instructions
                            if l2 and l2[-1] is nop:
                                l2.pop()
                                break
                        nop.sync_info = mybir.SyncInfo(on_wait=[w],
                                                       on_update=[])
                        new.append(nop)
                    inst.sync_info = mybir.SyncInfo(
                        on_wait=[waits[-1]],
                        on_update=list(si.on_update or []))
                    changed = True
                new.append(inst)
            if changed:
                live[:] = new


def build_nc():
    nc = bass.Bass()

    hidden = nc.declare_dram_parameter("hidden", [B, 4096], F32, isOutput=False)
    kp = nc.declare_dram_parameter("kp", [B, S, D], F32, isOutput=False)
    vp = nc.declare_dram_parameter("vp", [B, S, D], F32, isOutput=False)
    wq = nc.declare_dram_parameter("wq", [4096, NH * D], BF16, isOutput=False)
    wk = nc.declare_dram_parameter("wk", [4096, D], BF16, isOutput=False)
    wv = nc.declare_dram_parameter("wv", [4096, D], BF16, isOutput=False)
    wo = nc.declare_dram_parameter("wo", [NH * D, 4096], BF16, isOutput=False)
    pos = nc.declare_dram_parameter("pos", [1, B], F32, isOutput=False)
    ident = nc.declare_dram_parameter("ident", [128, 128], F32, isOutput=False)
    invf = nc.declare_dram_parameter("invf", [128, 1], F32, isOutput=False)
    sgn = nc.declare_dram_parameter("sgn", [128, 1], F32, isOutput=False)
    out_d = nc.declare_dram_parameter("out", [B, 4096], F32, isOutput=True)

    with tile.TileContext(nc) as tc:
        _emit(nc, tc, hidden, kp, vp, wq, wk, wv, wo, pos, ident, invf, sgn,
              out_d)
    _split_multi_waits(nc)
    return nc


def _emit(nc, tc, hidden, kp, vp, wq, wk, wv, wo, pos, ident, invf, sgn,
          out_d):
    from contextlib import ExitStack

    def eng(key):
        return nc.gpsimd if GPS_PASSES[key] else nc.vector

    with ExitStack() as ctx:
        ec = ctx.enter_context
        singles = ec(tc.tile_pool(name="singles", bufs=1))
        kcp = ec(tc.tile_pool(name="kcp", bufs=2))
        vcp = ec(tc.tile_pool(name="vcp", bufs=2))
        ktp = ec(tc.tile_pool(name="ktp", bufs=2))
        kzp = ec(tc.tile_pool(name="kzp", bufs=2))
        vzp = ec(tc.tile_pool(name="vzp", bufs=2))
        tails = ec(tc.tile_pool(name="tails", bufs=1))
        stats = ec(tc.tile_pool(name="stats", bufs=3))
        wpool = ec(tc.tile_pool(name="wpool", bufs=4))
        ptp = ec(tc.tile_pool(name="ptp", bufs=3))
        misc = ec(tc.tile_pool(name="misc", bufs=2))
        hstp = ec(tc.tile_pool(name="hstp", bufs=1))
        wop = ec(tc.tile_pool(name="wop", bufs=4))
        pstr = ec(tc.tile_pool(name="pstr", bufs=2, space="PSUM"))
        psatt = ec(tc.tile_pool(name="psatt", bufs=2, space="PSUM"))
        pspv = ec(tc.tile_pool(name="pspv", bufs=2, space="PSUM"))
        pssm = ec(tc.tile_pool(name="pssm", bufs=2, space="PSUM"))

        # ---- constants -------------------------------------------------
        ident_sb = singles.tile([128, 128], F32)
        nc.sync.dma_start(out=ident_sb[:], in_=ident[:])
        invf_sb = singles.tile([128, 1], F32)
        nc.sync.dma_start(out=invf_sb[:], in_=invf[:])
        sgn_sb = singles.tile([128, 1], F32)
        nc.sync.dma_start(out=sgn_sb[:], in_=sgn[:])
        posr = singles.tile([128, B], F32)
        nc.sync.dma_start(out=posr[:], in_=pos[:].to_broadcast((128, B)))
        zerob = singles.tile([128, 1], F32)
        nc.vector.memset(zerob[:], 0.0)
        ones1 = singles.tile([P1, 1], BF16)
        nc.vector.memset(ones1[:], 1.0)
        onesF = singles.tile([PF, 1], BF16)
        nc.vector.memset(onesF[:], 1.0)

        # ---- kick off b=0 cache loads before anything else -------------
        kcs = {}
        vcs = {}

        def load_k_wave(b, u):
            kc = kcp.tile([P1, E, D], F32, tag="kc")
            nc.sync.dma_start(
                out=kc[:],
                in_=kp[b, 0:NQ, :].rearrange("(p t) d -> p t d",
                                             p=P1)[:, E * u:E * (u + 1), :])
            return kc

        def load_v_wave(b, w):
            vc = vcp.tile([P1, E, D], F32, tag="vc")
            nc.sync.dma_start(
                out=vc[:],
                in_=vp[b, 0:NQ, :].rearrange("(p t) d -> p t d",
                                             p=P1)[:, E * w:E * (w + 1), :])
            return vc

        kcs[(0, 0)] = load_k_wave(0, 0)
        vcs[(0, 0)] = load_v_wave(0, 0)
        kcs[(0, 1)] = load_k_wave(0, 1)
        vcs[(0, 1)] = load_v_wave(0, 1)

        # ---- hidden^T --------------------------------------------------
        hT = singles.tile([128, 32, B], F32)
        for kk in range(0, 32, 8):
            hst = hstp.tile([B, 8 * 128], F32, tag="hst")
            nc.sync.dma_start(out=hst[:],
                              in_=hidden[:, 1024 * (kk // 8):
                                         1024 * (kk // 8 + 1)])
            ps_h = pssm.tile([128, 8 * B], F32, tag="sm")
            for j in range(8):
                nc.tensor.transpose(
                    ps_h[:, 4 * j:4 * j + 4],
                    hst[:, 128 * j:128 * (j + 1)],
                    ident_sb[0:B, 0:B],
                )
            nc.scalar.copy(hT[:, kk:kk + 8, :].rearrange("p k b -> p (k b)"),
                           ps_h[:])
        hTb = singles.tile([128, 32, B], BF16)
        nc.scalar.copy(hTb[:].rearrange("p k b -> p (k b)"),
                       hT[:].rearrange("p k b -> p (k b)"))

        # ---- projections (bf16, ACT-cast weights) ----------------------
        q_bh = singles.tile([B, NH * D], F32)
        k_bd = singles.tile([B, D], F32)
        v_new = singles.tile([B, D], F32)
        for w_d, n_cols, dst, wtag in ((wq, NH * D, q_bh, "wq"),
                                       (wk, D, k_bd, "wk"),
                                       (wv, D, v_new, "wv")):
            ps_p = pssm.tile([B, n_cols], F32, tag="sm")
            for k in range(32):
                w_t = wpool.tile([128, n_cols], BF16, tag=wtag)
                nc.sync.dma_start(out=w_t[:],
                                  in_=w_d[128 * k:128 * (k + 1), :])
                nc.tensor.matmul(ps_p[:], hTb[:, k, :], w_t[:],
                                 start=(k == 0), stop=(k == 31))
            nc.scalar.copy(dst[:], ps_p[:])
        v_new_f = singles.tile([1, B, D], F32)
        for bb in range(B):
            nc.sync.dma_start(out=v_new_f[0:1, bb, :],
                              in_=v_new[bb:bb + 1, :])
        v_new_b = singles.tile([1, B, D], BF16)
        nc.scalar.copy(v_new_b[0:1, :, :].rearrange("p b d -> p (b d)"),
                       v_new_f[0:1, :, :].rearrange("p b d -> p (b d)"))

        ps_qT = pssm.tile([128, NH * B], F32, tag="sm")
        for h in range(NH):
            nc.tensor.transpose(ps_qT[:, 4 * h:4 * h + 4],
                                q_bh[:, 128 * h:128 * (h + 1)],
                                ident_sb[0:B, 0:B])
        qT = singles.tile([128, NH, B], F32)
        nc.scalar.copy(qT[:].rearrange("p h b -> p (h b)"), ps_qT[:])
        ps_kT = pssm.tile([128, B], F32, tag="sm")
        nc.tensor.transpose(ps_kT[:], k_bd[:], ident_sb[0:B, 0:B])
        kT_new = singles.tile([128, B], F32)
        nc.scalar.copy(kT_new[:], ps_kT[:])

        # ---- RoPE ------------------------------------------------------
        fT = singles.tile([128, B], F32)
        nc.vector.tensor_mul(fT[:], posr[:], invf_sb[:].to_broadcast((128, B)))
        rk = singles.tile([128, B], F32)
        nc.vector.tensor_scalar(rk[:], fT[:], INV_2PI, None, OP.mult)
        nc.vector.tensor_scalar(rk[:], rk[:], MAGIC * 1.5, MAGIC * 1.5,
                                OP.add, OP.subtract)
        m1 = singles.tile([128, B], F32)
        nc.vector.scalar_tensor_tensor(m1[:], rk[:], -C1, fT[:],
                                       OP.mult, OP.add)
        nc.vector.scalar_tensor_tensor(m1[:], rk[:], -C2, m1[:],
                                       OP.mult, OP.add)
        sinT = singles.tile([128, B], F32)
        cosT = singles.tile([128, B], F32)
        nc.scalar.activation(sinT[:], m1[:], ACTF.Sin, bias=zerob[:])
        fc = singles.tile([128, B], F32)
        nc.vector.tensor_scalar(fc[:], fT[:], float(np.pi / 2), None, OP.add)
        rkc = singles.tile([128, B], F32)
        nc.vector.tensor_scalar(rkc[:], fc[:], INV_2PI, None, OP.mult)
        nc.vector.tensor_scalar(rkc[:], rkc[:], MAGIC * 1.5, MAGIC * 1.5,
                                OP.add, OP.subtract)
        mc = singles.tile([128, B], F32)
        nc.vector.scalar_tensor_tensor(mc[:], rkc[:], -C1, fc[:],
                                       OP.mult, OP.add)
        nc.vector.scalar_tensor_tensor(mc[:], rkc[:], -C2, mc[:],
                                       OP.mult, OP.add)
        nc.scalar.activation(cosT[:], mc[:], ACTF.Sin, bias=zerob[:])
        nc.vector.tensor_scalar(sinT[:], sinT[:], sgn_sb[:], None, OP.mult)

        qsw = singles.tile([128, NH, B], F32)
        nc.sync.dma_start(out=qsw[0:64], in_=qT[64:128])
        nc.sync.dma_start(out=qsw[64:128], in_=qT[0:64])
        ksw = singles.tile([128, B], F32)
        nc.sync.dma_start(out=ksw[0:64], in_=kT_new[64:128])
        nc.sync.dma_start(out=ksw[64:128], in_=kT_new[0:64])

        qR = singles.tile([128, NH, B], F32)
        nc.vector.tensor_mul(qR[:], qT[:], _bc(cosT[:], 1, NH))
        qs2 = singles.tile([128, NH, B], F32)
        nc.vector.tensor_mul(qs2[:], qsw[:], _bc(sinT[:], 1, NH))
        nc.vector.tensor_add(qR[:], qR[:], qs2[:])
        kR = singles.tile([128, B], F32)
        nc.vector.tensor_mul(kR[:], kT_new[:], cosT[:])
        ks2 = singles.tile([128, B], F32)
        nc.vector.tensor_mul(ks2[:], ksw[:], sinT[:])
        nc.vector.tensor_add(kR[:], kR[:], ks2[:])

        qbb = singles.tile([128, NH, B], BF16)
        nc.scalar.copy(qbb[:].rearrange("p h b -> p (h b)"),
                       qR[:].rearrange("p h b -> p (h b)"))
        kRb = singles.tile([128, B], BF16)
        nc.scalar.copy(kRb[:], kR[:])

        oTo = singles.tile([128, NH, B], BF16)

        for b in range(B):
            qb = qbb[:, :, b]

            # ======== V: two dequant half-waves (DVE work first) ========
            vz = vzp.tile([P1, T1, NH + D], BF16)
            for w in range(2):
                vc = vcs.pop((b, w), None)
                if vc is None:
                    vc = load_v_wave(b, w)
                vcv = vc[:].rearrange("p t (g e) -> p t g e", e=32)
                mnV = stats.tile([P1, E, 4], F32, tag="mnV")
                mxV = stats.tile([P1, E, 4], F32, tag="mxV")
                nc.vector.tensor_reduce(mnV[:], vcv, axis=AX.X, op=OP.min)
                nc.vector.tensor_reduce(mxV[:], vcv, axis=AX.X, op=OP.max)
                scV = stats.tile([P1, E, 4], F32, tag="scV")
                scv_f = scV[:].rearrange("p t g -> p (t g)")
                mnv_f = mnV[:].rearrange("p t g -> p (t g)")
                mxv_f = mxV[:].rearrange("p t g -> p (t g)")
                nc.vector.tensor_sub(scv_f, mxv_f, mnv_f)
                nc.vector.tensor_scalar(scv_f, scv_f, 1.0 / 3.0, None,
                                        OP.mult)
                invV = stats.tile([P1, E, 4], F32, tag="invV")
                invv_f = invV[:].rearrange("p t g -> p (t g)")
                nc.vector.reciprocal(invv_f, scv_f)
                vzw = vz[:, E * w:E * (w + 1), :]
                nc.scalar.copy(vzw[:, :, 0:NH], mnV[:])
                vcf = vc[:].rearrange("p t (g e) -> p (t g) e", e=32)
                eng("v1").tensor_sub(vcf, vcf, _bc(mnv_f, 2, 32))
                eng("v2").tensor_mul(vcf, vcf, _bc(invv_f, 2, 32))
                nc.vector.tensor_scalar(
                    vc[:].rearrange("p t d -> p (t d)"),
                    vc[:].rearrange("p t d -> p (t d)"),
                    MAGIC, None, OP.add)
                for g in range(4):
                    eng("v3").scalar_tensor_tensor(
                        vzw[:, :, NH + 32 * g:NH + 32 * (g + 1)],
                        vc[:, :, 32 * g:32 * (g + 1)], MAGIC,
                        _bc(scV[:, :, g], 2, 32), OP.subtract, OP.mult)
                nxt = (b, w + 1) if w == 0 else (b + 1, 0)
                if nxt[0] < B and nxt not in vcs:
                    vcs[nxt] = load_v_wave(*nxt)

            # ======== K: two half-waves ========
            kzT = kzp.tile([128, T1, P1], BF16)
            pa = psatt.tile([P1, T1 + 2, NH], F32)
            for u in range(2):
                kc = kcs.pop((b, u), None)
                if kc is None:
                    kc = load_k_wave(b, u)
                kt = ktp.tile([128, E, P1], F32)
                for g4 in range(8):
                    ptk = pstr.tile([128, 4, D], F32, tag="ptr")
                    for j in range(4):
                        t = 4 * g4 + j
                        nc.tensor.transpose(ptk[:, j, 0:P1], kc[:, t, :],
                                            ident_sb[0:P1, 0:P1])
                    nc.scalar.copy(kt[:, 4 * g4:4 * g4 + 4, :],
                                   ptk[:, :, 0:P1])
                # stats: group(2c+u) x d, reduce over e (= t within wave)
                ktv = kt[:].rearrange("p e c -> p c e")
                mnK = stats.tile([128, P1], F32, tag="mnK")
                mxK = stats.tile([128, P1], F32, tag="mxK")
                nc.vector.tensor_reduce(mnK[:], ktv, axis=AX.X, op=OP.min)
                nc.vector.tensor_reduce(mxK[:], ktv, axis=AX.X, op=OP.max)
                scK = stats.tile([128, P1], F32, tag="scK")
                nc.vector.tensor_sub(scK[:], mxK[:], mnK[:])
                nc.vector.tensor_scalar(scK[:], scK[:], 1.0 / 3.0, None,
                                        OP.mult)
                invK = stats.tile([128, P1], F32, tag="invK")
                nc.vector.reciprocal(invK[:], scK[:])
                mnKb = stats.tile([128, P1], BF16, tag="mnKb")
                nc.scalar.copy(mnKb[:], mnK[:])
                # dequant: y=x-mn; t=y*inv; r'=t+2^23 (RNE); z=(r'-M)*sc
                eng("k1").tensor_sub(kt[:], kt[:], _bc(mnK[:], 1, E))
                eng("k2").tensor_mul(kt[:], kt[:], _bc(invK[:], 1, E))
                nc.vector.tensor_scalar(
                    kt[:].rearrange("p e c -> p (e c)"),
                    kt[:].rearrange("p e c -> p (e c)"),
                    MAGIC, None, OP.add)
                eng("k3").scalar_tensor_tensor(
                    kzT[:, E * u:E * (u + 1), :], kt[:], MAGIC,
                    _bc(scK[:], 1, E), OP.subtract, OP.mult)
                # prefetch next wave's load
                nxt = (b, u + 1) if u == 0 else (b + 1, 0)
                if nxt[0] < B and nxt not in kcs:
                    kcs[nxt] = load_k_wave(*nxt)
                # scores for this half + group-mean fold
                for t in range(E * u, E * (u + 1)):
                    nc.tensor.matmul(pa[:, t, :], kzT[:, t, :], qb,
                                     start=True, stop=True)
                nc.tensor.matmul(pa[:, T1 + u, :], mnKb[:], qb,
                                 start=True, stop=True)

            # ======== K tail ========
            kcF = tails.tile([PF, TF, D], F32, tag="kcF")
            nc.sync.dma_start(
                out=kcF[:],
                in_=kp[b, NQ:S, :].rearrange("(p t) d -> p t d", p=PF))
            kTF = tails.tile([128, TF, D], BF16, tag="kTF")
            for g4 in range(2):
                ptf = pstr.tile([128, 4, D], F32, tag="ptr")
                for j in range(4):
                    t = 4 * g4 + j
                    nc.tensor.transpose(ptf[:, j, :], kcF[:, t, :],
                                        ident_sb[:])
                nc.scalar.copy(
                    kTF[:, 4 * g4:4 * g4 + 4, :].rearrange(
                        "p t c -> p (t c)"),
                    ptf[:].rearrange("p t c -> p (t c)"))
            paf = pssm.tile([PF, TF + 1, NH], F32, tag="sm")
            for t in range(TF):
                nc.tensor.matmul(paf[:, t, :], kTF[:, t, :], qb,
                                 start=True, stop=True)
            nc.tensor.matmul(paf[0:1, TF, :], kRb[:, b:b + 1], qb,
                             start=True, stop=True)

            # ======== softmax numerators ========
            pmns = misc.tile([P1, 2, NH], F32, tag="pmns")
            nc.scalar.copy(pmns[:].rearrange("p u h -> p (u h)"),
                           pa[:, T1:T1 + 2, :].rearrange("p u h -> p (u h)"))
            for u in range(2):
                pah = pa[:, E * u:E * (u + 1), :]
                nc.vector.tensor_add(pah, pah, _bc(pmns[:, u, :], 1, E))
            pT = ptp.tile([P1, T1, NH], BF16, tag="pT")
            nc.scalar.activation(pT[:].rearrange("p t h -> p (t h)"),
                                 pa[:, 0:T1, :].rearrange("p t h -> p (t h)"),
                                 ACTF.Exp, bias=zerob[0:P1, :],
                                 scale=INV_SQRT_D)
            pTf = ptp.tile([PF, TF, NH], BF16, tag="pTf")
            nc.scalar.activation(pTf[:].rearrange("p t h -> p (t h)"),
                                 paf[:, 0:TF, :].rearrange("p t h -> p (t h)"),
                                 ACTF.Exp, bias=zerob[0:PF, :],
                                 scale=INV_SQRT_D)
            pTn = ptp.tile([1, NH], BF16, tag="pTn")
            nc.scalar.activation(pTn[:], paf[0:1, TF, :], ACTF.Exp,
                                 bias=zerob[0:1, :], scale=INV_SQRT_D)

            # ======== denominators ========
            psd = pssm.tile([1, (T1 + TF + 1) * NH], F32, tag="sm")
            nc.tensor.matmul(psd[:, 0:T1 * NH], ones1[:],
                             pT[:].rearrange("p t h -> p (t h)"),
                             start=True, stop=True)
            nc.tensor.matmul(psd[:, T1 * NH:(T1 + TF) * NH], onesF[:],
                             pTf[:].rearrange("p t h -> p (t h)"),
                             start=True, stop=True)
            nc.tensor.matmul(psd[:, (T1 + TF) * NH:], ones1[0:1, :], pTn[:],
                             start=True, stop=True)
            stot = misc.tile([1, NH], F32, tag="stot")
            nc.vector.tensor_reduce(
                stot[:],
                psd[:].rearrange("p (t h) -> p h t", h=NH),
                axis=AX.X, op=OP.add)
            pst = pssm.tile([NH, 1], F32, tag="sm")
            nc.tensor.transpose(pst[:], stot[:], ident_sb[0:1, 0:1])
            rsc = misc.tile([NH, 1], F32, tag="rsc")
            nc.vector.reciprocal(rsc[:], pst[:])

            # ======== PV ========
            pv = pspv.tile([NH, NH + D], F32, tag="pv")
            for t in range(T1):
                nc.tensor.matmul(pv[:], pT[:, t, :], vz[:, t, :],
                                 start=(t == 0), stop=False)

            # ======== V tail + PV tail ========
            vcF = tails.tile([PF, TF, D], F32, tag="vcF")
            nc.sync.dma_start(
                out=vcF[:],
                in_=vp[b, NQ:S, :].rearrange("(p t) d -> p t d", p=PF))
            vFb = tails.tile([PF, TF, D], BF16, tag="vFb")
            nc.scalar.copy(vFb[:].rearrange("p t d -> p (t d)"),
                           vcF[:].rearrange("p t d -> p (t d)"))
            for t in range(TF):
                nc.tensor.matmul(pv[:, NH:], pTf[:, t, :], vFb[:, t, :],
                                 start=False, stop=False)
            nc.tensor.matmul(pv[:, NH:], pTn[:], v_new_b[0:1, b, :],
                             start=False, stop=True)
            pvmn = misc.tile([NH, NH], F32, tag="pvmn")
            nc.scalar.copy(pvmn[:], pv[:, 0:NH])
            ob = misc.tile([NH, D], F32, tag="ob")
            nc.vector.scalar_tensor_tensor(
                ob[:].rearrange("h (g e) -> h g e", e=32),
                pv[:, NH:].rearrange("h (g e) -> h g e", e=32),
                0.0, _bc(pvmn[:], 2, 32), OP.add, OP.add)
            obs = misc.tile([NH, D], F32, tag="obs")
            nc.scalar.activation(obs[:], ob[:], ACTF.Copy, scale=rsc[:])
            pso2 = pssm.tile([128, NH], F32, tag="sm")
            nc.tensor.transpose(pso2[:], obs[:], ident_sb[0:NH, 0:NH])
            nc.scalar.copy(oTo[:, :, b], pso2[:])

            # ---- o_proj for this batch row (overlaps later batches) ----
            for nch in range(8):
                pso = pspv.tile([B, 512], F32, tag="pv")
                for h in range(NH):
                    wo_t = wop.tile([128, 512], BF16, tag="wo")
                    nc.sync.dma_start(
                        out=wo_t[:],
                        in_=wo[128 * h:128 * (h + 1),
                               512 * nch:512 * (nch + 1)])
                    nc.tensor.matmul(pso[0:1, :], oTo[:, h, b:b + 1], wo_t[:],
                                     start=(h == 0), stop=(h == NH - 1))
                outp = misc.tile([B, 512], F32, tag="outp")
                nc.scalar.copy(outp[0:1, :], pso[0:1, :])
                nc.sync.dma_start(
                    out=out_d[b:b + 1, 512 * nch:512 * (nch + 1)],
                    in_=outp[0:1, :])




# ----------------------------------------------------------------------
_NC = None


def _get_nc():
    global _NC
    if _NC is None:
        _NC = build_nc()
    return _NC


def _host_consts():
    ident = np.eye(128, dtype=np.float32)
    inv_freq = (1.0 / (np.float32(10000.0) **
                       (np.arange(0, D, 2).astype(np.float32) / np.float32(D))))
    invf = np.tile(inv_freq.astype(np.float32), 2).reshape(128, 1)
    sgn = np.concatenate([-np.ones(64, np.float32),
                          np.ones(64, np.float32)]).reshape(128, 1)
    return ident, invf, sgn


def _make_in_maps(hidden_states, key_past, value_past, wq, wk, wv, wo,
                  position_ids, past_len=None):
    ident, invf, sgn = _host_consts()
    pos_f = np.asarray(position_ids).astype(np.float32).reshape(1, B)
    hid = np.ascontiguousarray(
        np.asarray(hidden_states, np.float32).reshape(B, 4096))
    in_maps = []
    for c in range(8):
        in_maps.append({
            "hidden": hid,
            "kp": np.ascontiguousarray(np.asarray(key_past, np.float32)[:, c]),
            "vp": np.ascontiguousarray(np.asarray(value_past, np.float32)[:, c]),
            "wq": np.ascontiguousarray(np.asarray(wq, np.float32)[:, 512 * c:512 * (c + 1)].astype(_BF)),
            "wk": np.ascontiguousarray(np.asarray(wk, np.float32)[:, 128 * c:128 * (c + 1)].astype(_BF)),
            "wv": np.ascontiguousarray(np.asarray(wv, np.float32)[:, 128 * c:128 * (c + 1)].astype(_BF)),
            "wo": np.ascontiguousarray(np.asarray(wo, np.float32)[512 * c:512 * (c + 1), :].astype(_BF)),
            "pos": pos_f,
            "ident": ident,
            "invf": invf,
            "sgn": sgn,
        })
    return in_maps


def kernel(hidden_states, key_past, value_past, wq, wk, wv, wo, position_ids,
           past_len):
    nc = _get_nc()
    in_maps = _make_in_maps(hidden_states, key_past, value_past, wq, wk, wv,
                            wo, position_ids)
    res = run_bass_kernel_spmd(nc, in_maps, list(range(8)))
    out = np.zeros((B, 4096), np.float32)
    for r in res.results:
        out = out + r["out"]
    return out.reshape(B, 1, 4096)
